# revision 1
# baseline (speedup 1.0000x reference)
import numpy as np

# nn_CTRGraphBlock: B,C,Co,T,V,S,R,G = 64,128,128,256,25,3,16,32
# Sharding: data-parallel over batch B across the 8 NeuronCores
# (adjacency/alpha/conv weights replicated), per the sharding hint.
B, C, Co, T, V, S, R, G = 64, 128, 128, 256, 25, 3, 16, 32
EPS = 1e-5
N_CORES = 8


def _block(x, Wq, bq, Wk, bk, Wv, bv, Wr, br, A, alpha, gn_w, gn_b):
    import jax
    import jax.numpy as jnp

    xm = x.mean(axis=2)                                          # [b,C,V]
    q = jnp.einsum('bcv,src->bsrv', xm, Wq) + bq[None, :, :, None]
    k = jnp.einsum('bcv,src->bsrv', xm, Wk) + bk[None, :, :, None]
    v = jnp.einsum('bctv,soc->bsotv', x, Wv) + bv[None, :, :, None, None]
    rel = jnp.tanh(q[..., :, None] - k[..., None, :])            # [b,S,R,V,V]
    relc = jnp.einsum('bsruv,sor->bsouv', rel, Wr) + br[None, :, :, None, None]
    relc = relc * alpha[0] + A[None, :, None, :, :]              # [b,S,Co,V,V]
    out = jnp.einsum('bsouv,bsotv->botu', relc, v)               # [b,Co,T,V]
    b_ = x.shape[0]
    o = out.reshape(b_, G, Co // G, T, V)
    mu = o.mean(axis=(2, 3, 4), keepdims=True)
    var = ((o - mu) ** 2).mean(axis=(2, 3, 4), keepdims=True)
    o = ((o - mu) * jax.lax.rsqrt(var + EPS)).reshape(b_, Co, T, V)
    o = o * gn_w[None, :, None, None] + gn_b[None, :, None, None]
    return jax.nn.relu(o + x)


def _run_sharded(inputs):
    """Data-parallel over B across 8 neuron cores via pmap."""
    import jax

    devs = jax.devices()[:N_CORES]
    assert len(devs) == N_CORES
    x = inputs["x"]
    b = x.shape[0]
    per = b // N_CORES
    x_sh = x.reshape(N_CORES, per, *x.shape[1:])

    wnames = ["Wq", "bq", "Wk", "bk", "Wv", "bv", "Wr", "br", "A", "alpha",
              "gn_w", "gn_b"]
    fn = jax.pmap(
        lambda xs, *w: _block(xs, *w),
        in_axes=(0,) + (None,) * len(wnames),
        devices=devs,
    )
    out = fn(x_sh, *[inputs[n] for n in wnames])
    out = np.asarray(out).reshape(b, *out.shape[2:])
    return out.astype(np.float32)


def _run_local(inputs):
    """Fallback: single-device / CPU computation (still correct)."""
    import jax
    import jax.numpy as jnp

    args = {k: jnp.asarray(v) for k, v in inputs.items()}
    out = jax.jit(_block)(args["x"], args["Wq"], args["bq"], args["Wk"],
                          args["bk"], args["Wv"], args["bv"], args["Wr"],
                          args["br"], args["A"], args["alpha"], args["gn_w"],
                          args["gn_b"])
    return np.asarray(out, dtype=np.float32)


def kernel(**inputs) -> np.ndarray:
    inputs = {k: np.asarray(v, dtype=np.float32) for k, v in inputs.items()}
    try:
        return _run_sharded(inputs)
    except Exception:
        return _run_local(inputs)
